# revision 6
# baseline (speedup 1.0000x reference)
"""Trainium2 Bass kernel for nn_DressedQuantumNet (262144 x 64 -> 262144 x 1).

Math reduction (host, params only): the 4-qubit circuit after the per-sample
input RY layer is a FIXED 16x16 linear map U (depends only on q_params).
With the product state psi_w = (cos phi_w, sin phi_w),
phi_w = (pi/4)*(tanh(u_w)+1), u = x @ pre_w.T + pre_b:

    out = psi^T O psi + post_b,     O = sum_w post_w[w] * U^T Z_w U.

Device pipeline per core (32768 samples), fp16 data paths / fp32 psum:
  DMA-cast x->fp16 -> PE transpose -> pre-matmul (K=128: 2 sample-groups x
  64 feats, M=32 zero-padded) -> tanh(+pre_b) on ACT -> PE transpose
  (sample-major regroup) -> sin x2 on ACT -> pair/state products -> PE
  transpose -> block-diag O matmul -> elementwise dot -> reduce-matmul ->
  output transpose -> contiguous DMA out.

Sample bookkeeping: sample s = 8192*m + 64*p + k, k = 32*ut + 8*v + 2*cc + j.
Sample-major coords: s = 8192*(Q//64) + 64*mu + (Q%64) with mu=p,
Q = 64*m + k. Back half: Q = 32*w + 8*eb + qt.
"""
import sys

import numpy as np

for _p in ("/opt/trn_rl_repo",):
    if _p not in sys.path:
        sys.path.insert(0, _p)

import concourse.bass as bass
import concourse.bacc as bacc
import concourse.mybir as mybir
from concourse import tile
from concourse.bass_utils import run_bass_kernel_spmd

AF = mybir.ActivationFunctionType
ALU = mybir.AluOpType
F32 = mybir.dt.float32
F16 = mybir.dt.float16

N_CORES = 8
BATCH = 262144
S = BATCH // N_CORES          # 32768 samples per core
NM = S // 8192                # 4 macro-tiles per core
N_QUBITS = 4
Q_DEPTH = 6
IN_F = 64

TRACE = False                 # test.py sets True to collect a profile
LAST_RESULTS = None

# ---------------------------------------------------------------- host math


def _ry(theta):
    c, s = np.cos(theta / 2), np.sin(theta / 2)
    return np.array([[c, -s], [s, c]], dtype=np.float64)


def _lift1(gate, wire):
    ops = [np.eye(2)] * N_QUBITS
    ops[wire] = gate
    out = ops[0]
    for o in ops[1:]:
        out = np.kron(out, o)
    return out


def _cnot(ctrl, tgt):
    U = np.zeros((16, 16))
    for i in range(16):
        bits = [(i >> (N_QUBITS - 1 - w)) & 1 for w in range(N_QUBITS)]
        if bits[ctrl] == 1:
            bits[tgt] ^= 1
        j = sum(b << (N_QUBITS - 1 - w) for w, b in enumerate(bits))
        U[j, i] = 1.0
    return U


def quad_form(q_params, post_w):
    """O (16x16 fp64): out = psi^T O psi + post_b."""
    qw = np.asarray(q_params, dtype=np.float64).reshape(Q_DEPTH, N_QUBITS)
    U = np.eye(16)
    for k in range(Q_DEPTH):
        U = _cnot(0, 1) @ U
        U = _cnot(2, 3) @ U
        U = _cnot(1, 2) @ U
        for w in range(N_QUBITS):
            U = _lift1(_ry(qw[k, w]), w) @ U
    Z = np.diag([1.0, -1.0])
    O = np.zeros((16, 16))
    pw = np.asarray(post_w, dtype=np.float64).reshape(-1)
    for w in range(N_QUBITS):
        O += pw[w] * (U.T @ _lift1(Z, w) @ U)
    return O


def _consts(pre_w, pre_b, q_params, post_w):
    # Wstack (128, 32) f16: [64j + f, 4j + i] = pre_w[i, f]; rest zero.
    wstack = np.zeros((128, 32), dtype=np.float32)
    for j in range(2):
        for i in range(4):
            wstack[64 * j:64 * j + 64, 4 * j + i] = pre_w[i, :]
    # bias (128, 1) f32: row r -> pre_b[r % 4]
    biast = np.tile(np.asarray(pre_b, np.float32).reshape(4), 32)[:, None]
    biast = np.ascontiguousarray(biast, dtype=np.float32)
    # Mbd (128, 128) f16: blockdiag 8 x O (O symmetric)
    O = quad_form(q_params, post_w)
    mbd = np.zeros((128, 128), dtype=np.float32)
    for g in range(8):
        mbd[16 * g:16 * g + 16, 16 * g:16 * g + 16] = O
    # Rsum (128, 32) f16: [16q + st, q] = 1 for q in [0,8); rest zero.
    rsum = np.zeros((128, 32), dtype=np.float32)
    for q in range(8):
        rsum[16 * q:16 * q + 16, q] = 1.0
    ident16 = np.eye(128, dtype=np.float16)
    ident32 = np.eye(128, dtype=np.float32)
    return (wstack.astype(np.float16), biast, mbd.astype(np.float16),
            rsum.astype(np.float16), ident16, ident32)


# ---------------------------------------------------------------- program


def build(nm=NM, post_b=0.0):
    nc = bacc.Bacc()
    s_core = nm * 8192
    NQ = nm * 64                       # Q-column count (= s_core / 128)

    x = nc.declare_dram_parameter("x", (s_core, IN_F), F32, isOutput=False)
    y = nc.declare_dram_parameter("y", (s_core, 1), F32, isOutput=True)
    wstack_d = nc.declare_dram_parameter("wstack", (128, 32), F16, isOutput=False)
    bias_d = nc.declare_dram_parameter("biast", (128, 1), F32, isOutput=False)
    mbd_d = nc.declare_dram_parameter("mbd", (128, 128), F16, isOutput=False)
    rsum_d = nc.declare_dram_parameter("rsum", (128, 32), F16, isOutput=False)
    id16_d = nc.declare_dram_parameter("ident16", (128, 128), F16, isOutput=False)
    id32_d = nc.declare_dram_parameter("ident32", (128, 128), F32, isOutput=False)

    with tile.TileContext(nc) as tc:
        with (
            tc.tile_pool(name="const", bufs=1) as cpool,
            tc.tile_pool(name="xin", bufs=2) as xpool,
            tc.tile_pool(name="sb16", bufs=3) as spool,
            tc.tile_pool(name="pers", bufs=1) as ppool,
            tc.tile_pool(name="psxt", bufs=2, space="PSUM") as ps_xt,
            tc.tile_pool(name="ps16", bufs=2, space="PSUM") as ps_16,
            tc.tile_pool(name="psu", bufs=2, space="PSUM") as ps_u,
            tc.tile_pool(name="psmy", bufs=2, space="PSUM") as ps_my,
        ):
            # constants
            wstack = cpool.tile([128, 32], F16, tag="wstack")
            biast = cpool.tile([128, 1], F32, tag="biast")
            mbd = cpool.tile([128, 128], F16, tag="mbd")
            rsum = cpool.tile([128, 32], F16, tag="rsum")
            id16 = cpool.tile([128, 128], F16, tag="id16")
            id32 = cpool.tile([128, 128], F32, tag="id32")
            nc.sync.dma_start(wstack[:], wstack_d[:])
            nc.sync.dma_start(biast[:], bias_d[:])
            nc.sync.dma_start(mbd[:], mbd_d[:])
            nc.sync.dma_start(rsum[:], rsum_d[:])
            nc.sync.dma_start(id16[:], id16_d[:])
            nc.sync.dma_start(id32[:], id32_d[:])
            b_pi4 = cpool.tile([128, 1], F32, tag="b_pi4")
            b_3pi4 = cpool.tile([128, 1], F32, tag="b_3pi4")
            nc.gpsimd.memset(b_pi4[:], float(np.pi / 4))
            nc.gpsimd.memset(b_3pi4[:], float(3 * np.pi / 4))

            # persistent sample-major staging
            Tg = ppool.tile([128, 4 * NQ], F16, tag="tg")      # [mu, NQ*i + Q]
            Sg = ppool.tile([128, 4 * NQ], F16, tag="sg")
            Cg = ppool.tile([128, 4 * NQ], F16, tag="cg")
            PSI = ppool.tile([128, 16 * NQ], F16, tag="psi")   # [mu, 16Q + st]
            P01 = [ppool.tile([128, NQ], F16, tag=f"p01_{k}", name=f"P01_{k}") for k in range(4)]
            P23 = [ppool.tile([128, NQ], F16, tag=f"p23_{k}", name=f"P23_{k}") for k in range(4)]
            n_t = max(nm // 2, 1)
            Yo_s = [ppool.tile([128, 512], F32, tag=f"yos{t}", name=f"Yo_s{t}") for t in range(n_t)]
            Yo2 = [ppool.tile([128, 512], F32, tag=f"yo2{t}", name=f"Yo2_{t}") for t in range(n_t)]

            x_r = x[:].rearrange("(m p k) f -> m p (k f)", m=nm, p=128, k=64)

            for m in range(nm):
                X = xpool.tile([128, 4096], F16, tag="x")
                nc.gpsimd.dma_start(X[:], x_r[m])     # SWDGE f32->f16 cast

                for ut in range(2):
                    U = ps_u.tile([128, 512], F32, tag="u")
                    T = spool.tile([128, 512], F16, tag="t")
                    for v in range(4):
                        XTp = ps_xt.tile([128, 512], F16, tag="xt")
                        XTs = spool.tile([128, 512], F16, tag="xts")
                        for cc in range(4):
                            c = 16 * ut + 4 * v + cc
                            nc.tensor.transpose(
                                XTp[:, 128 * cc:128 * cc + 128],
                                X[:, 128 * c:128 * c + 128],
                                id16[:],
                            )
                        if v % 2 == 0:
                            nc.vector.tensor_copy(XTs[:], XTp[:])
                        else:
                            nc.scalar.copy(XTs[:], XTp[:])
                        nc.tensor.matmul(
                            U[32 * v:32 * v + 32, :], wstack[:], XTs[:],
                            tile_position=(0, 32 * v),
                        )
                    # t = tanh(u + pre_b); zero-padded rows produce junk that
                    # is skipped downstream.
                    nc.scalar.activation(T[:], U[:], AF.Tanh, bias=biast[:])

                    # regroup: PE-transpose T chunks, compact-copy into Tg
                    P2 = ps_16.tile([128, 512], F16, tag="p16")
                    for cc in range(4):
                        nc.tensor.transpose(
                            P2[:, 128 * cc:128 * cc + 128],
                            T[:, 128 * cc:128 * cc + 128],
                            id16[:],
                        )
                    # P2[mu, 128cc + 32v + 4j + i] -> Tg[mu, NQ*i + Q],
                    # Q = 64m + 32ut + 8v + 2cc + j
                    p2r = P2[:].rearrange("p (c v x) -> p c v x", c=4, v=4, x=32)
                    tgr = Tg[:].rearrange(
                        "p (i mm uu vv cc jj) -> p cc vv i mm uu jj",
                        i=4, mm=nm, uu=2, vv=4, cc=4, jj=2)
                    for j in range(2):
                        src = p2r[:, :, :, 4 * j:4 * j + 4]       # (128,4,4,4)
                        dst = tgr[:, :, :, :, m, ut, j]           # (128,4,4,4)
                        nc.vector.tensor_copy(dst, src)

                # sins for this macro's Q range [64m, 64m+64)
                tg_m = Tg[:].rearrange("p (i q) -> p i q", i=4)[:, :, 64 * m:64 * m + 64]
                sg_m = Sg[:].rearrange("p (i q) -> p i q", i=4)[:, :, 64 * m:64 * m + 64]
                cg_m = Cg[:].rearrange("p (i q) -> p i q", i=4)[:, :, 64 * m:64 * m + 64]
                nc.scalar.activation(sg_m, tg_m, AF.Sin,
                                     bias=b_pi4[:], scale=float(np.pi / 4))
                nc.scalar.activation(cg_m, tg_m, AF.Sin,
                                     bias=b_3pi4[:], scale=float(np.pi / 4))

                # products: P01[2*i0+i1] = a0(i0)*a1(i1), a(0)=cos, a(1)=sin
                q0, q1 = 64 * m, 64 * m + 64
                aw = []
                for w in range(4):
                    aw.append([Cg[:, w * NQ + q0:w * NQ + q1],
                               Sg[:, w * NQ + q0:w * NQ + q1]])
                for i0 in range(2):
                    for i1 in range(2):
                        nc.gpsimd.tensor_tensor(
                            P01[2 * i0 + i1][:, q0:q1], aw[0][i0], aw[1][i1],
                            ALU.mult)
                        nc.gpsimd.tensor_tensor(
                            P23[2 * i0 + i1][:, q0:q1], aw[2][i0], aw[3][i1],
                            ALU.mult)
                psi_m = PSI[:].rearrange("p (q s) -> p q s", s=16)[:, q0:q1, :]
                for st in range(16):
                    nc.gpsimd.tensor_tensor(
                        psi_m[:, :, st], P01[st >> 2][:, q0:q1],
                        P23[st & 3][:, q0:q1], ALU.mult)

                # back half: PSIT banks w = 2m, 2m+1
                for wl in range(2):
                    w = 2 * m + wl
                    PSIT = ps_16.tile([128, 512], F16, tag="p16")
                    PSIT_s = spool.tile([128, 512], F16, tag="psts")
                    for eb in range(4):
                        ep = 4 * w + eb
                        nc.tensor.transpose(
                            PSIT[:, 128 * eb:128 * eb + 128],
                            PSI[:, 128 * ep:128 * ep + 128],
                            id16[:],
                        )
                    nc.vector.tensor_copy(PSIT_s[:], PSIT[:])
                    MP = ps_my.tile([128, 512], F32, tag="mp")
                    nc.tensor.matmul(MP[:], mbd[:], PSIT_s[:])
                    DP = spool.tile([128, 512], F16, tag="dp")
                    nc.vector.tensor_tensor(DP[:], PSIT_s[:], MP[:], ALU.mult)
                    Yp = ps_my.tile([128, 512], F32, tag="mp")
                    nc.tensor.matmul(Yp[0:32, :], rsum[:], DP[:],
                                     tile_position=(0, 0))
                    t_idx, w_loc = w // 4, w % 4
                    nc.scalar.activation(
                        Yo_s[t_idx][32 * w_loc:32 * w_loc + 32, :],
                        Yp[0:32, :], AF.Copy, bias=float(post_b))

            # output fix-up transpose + store
            for t in range(n_t):
                YT = ps_my.tile([128, 512], F32, tag="mp")
                for eb in range(4):
                    nc.tensor.transpose(
                        YT[:, 128 * eb:128 * eb + 128],
                        Yo_s[t][:, 128 * eb:128 * eb + 128],
                        id32[:],
                    )
                nc.vector.tensor_copy(Yo2[t][:], YT[:])
                # Yo2[mu, 128eb + 32w + qt] = y[16384t + 8192wh + 64mu
                #                               + 32wl + 8eb + qt], w = 2wh+wl
                src_r = Yo2[t][:].rearrange(
                    "p (e wh wl q) -> p wh wl e q", e=4, wh=2, wl=2, q=32
                )[:, :, :, :, 0:8]
                dst_r = y[:].rearrange(
                    "(tt wh mu wl e q) o -> tt wh wl mu e (q o)",
                    tt=n_t, wh=2, mu=128, wl=2, e=4, q=8)[t]
                for wh in range(2):
                    for wl in range(2):
                        nc.sync.dma_start(dst_r[wh, wl], src_r[:, wh, wl])

    return nc


# ---------------------------------------------------------------- entry


def kernel(input_features, pre_w, pre_b, q_params, post_w, post_b):
    global LAST_RESULTS
    x_full = np.ascontiguousarray(np.asarray(input_features, np.float32))
    wst, biast, mbd, rsum, id16, id32 = _consts(
        np.asarray(pre_w, np.float32), np.asarray(pre_b, np.float32),
        np.asarray(q_params, np.float32), np.asarray(post_w, np.float32))
    post_b_f = float(np.asarray(post_b).reshape(-1)[0])

    nc = build(nm=NM, post_b=post_b_f)

    shards = x_full.reshape(N_CORES, S, IN_F)
    in_maps = [
        dict(x=np.ascontiguousarray(shards[c]), wstack=wst, biast=biast,
             mbd=mbd, rsum=rsum, ident16=id16, ident32=id32)
        for c in range(N_CORES)
    ]
    nc.finalize()
    res = run_bass_kernel_spmd(nc, in_maps, list(range(N_CORES)), trace=TRACE)
    LAST_RESULTS = res
    out = np.concatenate([np.asarray(r["y"]).reshape(S, 1) for r in res.results])
    return out.astype(np.float32)


if __name__ == "__main__":
    print("kernel module OK")
